# revision 29
# baseline (speedup 1.0000x reference)
"""GATv2 message-passing layer on 8 Trainium2 NeuronCores.

Strategy (see spec sharding_hint):
  - Host: sort/pack edges by destination node into 128-node "blocks"
    (greedy balanced bin-packing with a host-side node permutation so
    every block has <= CPB*128 in-edges), shard blocks across 8 cores.
  - Device phase A (all cores, redundant): LayerNorm+ReLU+xl linear for
    ALL nodes -> xl table [N, 258] in local DRAM (row layout
    [xl_h0(128) | 1.0 | xl_h1(128) | 1.0] so segment-sum matmuls get the
    softmax denominator for free from the ones column).
  - Device phase B (local slots): gather x[perm], LayerNorm+ReLU,
    xr linear (+ones cols) -> xr_local, residual base (x + h@res_W + bias)
    kept in SBUF.
  - Device phase 2 (per block): indirect-DMA gather of xl[src] rows for
    the block's edges, selection-matrix matmuls to broadcast xr[dst] to
    edges and to segment-sum alpha-weighted messages + softmax
    normalizers, LeakyReLU+attention dot on ACT/DVE, exp, normalize,
    add residual, write out.
  - Host: inverse-permute the per-core outputs into the full [N, D] result.
"""

import os
import sys

sys.path.insert(0, "/opt/trn_rl_repo")

import numpy as np

import concourse.bass as bass
import concourse.tile as tile
from concourse import mybir
from concourse.bass import IndirectOffsetOnAxis
from concourse.bass_utils import run_bass_kernel_spmd

F32 = mybir.dt.float32
I32 = mybir.dt.int32

N_CORES = 8
N = 100000
D = 128
H = 2
P = 128
NEG_SLOPE = 0.2
NB = 98                 # blocks (of 128 dst nodes) per core
SLOTS = NB * P          # node slots per core (12544)
NTILES_A = (N + P - 1) // P   # 782 tiles in phase A (last has 32 rows)
# xl/xr table row: [h0(128) | 1 | h1(128) | 1 | q0 | q1] where q_h = att_h . xl_h
ROW = 2 * (D + 1) + 2   # 260


# ----------------------------------------------------------------------------
# Host-side graph packing
# ----------------------------------------------------------------------------

def _pack_graph(edge_index: np.ndarray):
    """Assign nodes to 784 blocks of <=128 nodes with balanced in-edge
    counts, then lay out each block's edges into (partition, chunk) slots.

    Returns (cpb, per-core dicts of gidx/dcol/pidx, host perm info).
    """
    src = np.ascontiguousarray(edge_index[0]).astype(np.int64)
    dst = np.ascontiguousarray(edge_index[1]).astype(np.int64)
    E = src.shape[0]
    nblk_total = N_CORES * NB

    deg = np.bincount(dst, minlength=N).astype(np.int64)

    # Greedy balanced packing: nodes in decreasing degree order into the
    # currently lightest (by edges) block that still has node capacity.
    order = np.argsort(-deg, kind="stable")
    import heapq

    heap = [(0, b) for b in range(nblk_total)]  # (edge load, block)
    heapq.heapify(heap)
    blk_nodes = [[] for _ in range(nblk_total)]
    blk_load = np.zeros(nblk_total, dtype=np.int64)
    spill = []
    for node in order:
        d_n = int(deg[node])
        while True:
            load, b = heapq.heappop(heap)
            if len(blk_nodes[b]) < P:
                blk_nodes[b].append(int(node))
                blk_load[b] = load + d_n
                heapq.heappush(heap, (load + d_n, b))
                break
            spill.append((load, b))  # full block: drop from heap
        # (dropped-full blocks are never pushed back)
    cpb = int(np.max(blk_load + P - 1) // P)
    cpb = max(cpb, 1)

    # node -> (block, position-in-block)
    node_blk = np.full(N, -1, dtype=np.int32)
    node_pos = np.full(N, -1, dtype=np.int32)
    for b in range(nblk_total):
        ns = blk_nodes[b]
        node_blk[ns] = b
        node_pos[ns] = np.arange(len(ns), dtype=np.int32)

    # Group edges by destination block, then assign slots t = j*128 + p in
    # arrival order.
    eblk = node_blk[dst]
    eorder = np.argsort(eblk, kind="stable")
    eblk_sorted = eblk[eorder]
    starts = np.searchsorted(eblk_sorted, np.arange(nblk_total))
    ends = np.searchsorted(eblk_sorted, np.arange(nblk_total) + 1)

    ecap = cpb * P
    gidx = np.zeros((nblk_total, P, cpb), dtype=np.int32)   # src node id
    dcol = np.full((nblk_total, P, cpb), 300.0, dtype=np.float32)
    for b in range(nblk_total):
        eids = eorder[starts[b]:ends[b]]
        ne = eids.shape[0]
        assert ne <= ecap, (b, ne, ecap)
        t = np.arange(ne)
        pp, jj = t % P, t // P
        gidx[b, pp, jj] = src[eids].astype(np.int32)
        dcol[b, pp, jj] = node_pos[dst[eids]].astype(np.float32)

    # Per-core slot permutation: slot s = g*128 + p -> node id (0 for pads)
    pidx = np.zeros((nblk_total, P), dtype=np.int32)
    valid = np.zeros((nblk_total, P), dtype=bool)
    for b in range(nblk_total):
        ns = blk_nodes[b]
        pidx[b, : len(ns)] = ns
        valid[b, : len(ns)] = True

    cores = []
    for c in range(N_CORES):
        bs = slice(c * NB, (c + 1) * NB)
        cores.append(
            dict(
                gidx=np.ascontiguousarray(gidx[bs]),
                dcol=np.ascontiguousarray(dcol[bs]),
                # device wants [p, g] layout: pidx[p, g] = slot g*128+p
                pidx=np.ascontiguousarray(pidx[bs].T.astype(np.int32)),
                pidx_host=pidx[bs],          # [NB, P]
                valid=valid[bs],             # [NB, P]
            )
        )
    return cpb, cores


# ----------------------------------------------------------------------------
# Device program
# ----------------------------------------------------------------------------

def _ln_relu(nc, pools, x_ap, r, use_gb, gb_g=None, gb_b=None, eps=None):
    """LayerNorm(+gamma/beta)+ReLU of x_ap[:r] ([128,128] tile) -> returns
    SBUF tile hr [128,128] (rows [:r] valid)."""
    stat, work = pools
    st = stat.tile([P, nc.vector.BN_STATS_DIM], F32, tag="ln_st")
    nc.vector.bn_stats(out=st[:r], in_=x_ap)
    mv = stat.tile([P, nc.vector.BN_AGGR_DIM], F32, tag="ln_mv")
    nc.vector.bn_aggr(out=mv[:r], in_=st[:r])
    rstd = stat.tile([P, 1], F32, tag="ln_rstd")
    nc.scalar.activation(
        out=rstd[:r], in_=mv[:r, 1:2],
        func=mybir.ActivationFunctionType.Sqrt, bias=eps[:r], scale=1.0,
    )
    nc.vector.reciprocal(out=rstd[:r], in_=rstd[:r])
    hh = work.tile([P, D], F32, tag="ln_hh")
    nc.vector.tensor_scalar(
        out=hh[:r], in0=x_ap, scalar1=mv[:r, 0:1], scalar2=rstd[:r],
        op0=mybir.AluOpType.subtract, op1=mybir.AluOpType.mult,
    )
    if use_gb:
        nc.vector.tensor_tensor(out=hh[:r], in0=hh[:r], in1=gb_g[:r],
                                op=mybir.AluOpType.mult)
        nc.vector.tensor_tensor(out=hh[:r], in0=hh[:r], in1=gb_b[:r],
                                op=mybir.AluOpType.add)
    hr = work.tile([P, D], F32, tag="ln_hr")
    nc.scalar.activation(out=hr[:r], in_=hh[:r],
                         func=mybir.ActivationFunctionType.Relu)
    return hr


def build_program(cpb, use_gb, use_bl, use_brb, mm_dt=F32):
    nc = bass.Bass()

    x_ext = nc.declare_dram_parameter("x", [N, D], F32, isOutput=False)
    wla = nc.declare_dram_parameter("wla", [D, ROW], F32, isOutput=False)
    wrb = nc.declare_dram_parameter("wrb", [D, ROW + D], F32, isOutput=False)
    attb = nc.declare_dram_parameter("attb", [P, 2 * D], F32, isOutput=False)
    iotac_d = nc.declare_dram_parameter("iotac", [P, 1], F32, isOutput=False)
    iotar_d = nc.declare_dram_parameter("iotar", [P, P], F32, isOutput=False)
    ident_d = nc.declare_dram_parameter("ident", [P, P], F32, isOutput=False)
    gidx_d = nc.declare_dram_parameter("gidx", [NB, P, cpb], I32, isOutput=False)
    dcol_d = nc.declare_dram_parameter("dcol", [NB, P, cpb], F32, isOutput=False)
    pidx_d = nc.declare_dram_parameter("pidx", [P, NB], I32, isOutput=False)
    out_ext = nc.declare_dram_parameter("out", [SLOTS, D], F32, isOutput=True)
    if use_gb:
        gbg_d = nc.declare_dram_parameter("gbg", [P, D], F32, isOutput=False)
        gbb_d = nc.declare_dram_parameter("gbb", [P, D], F32, isOutput=False)
    if use_bl:
        blb_d = nc.declare_dram_parameter("blb", [P, ROW], F32, isOutput=False)
    if use_brb:
        brb_d = nc.declare_dram_parameter("brb", [P, ROW + D], F32, isOutput=False)

    xlt = nc.dram_tensor("xlt", [N, ROW], F32)
    xrl = nc.dram_tensor("xrl", [SLOTS, ROW], F32)

    def mm(ap):
        return ap.bitcast(mm_dt) if mm_dt != F32 else ap

    with tile.TileContext(nc) as tc:
        with (
            tc.tile_pool(name="singles", bufs=1) as singles,
            tc.tile_pool(name="stat", bufs=4) as stat,
            tc.tile_pool(name="work", bufs=3) as work,
            tc.tile_pool(name="io", bufs=3) as io,
        ):
            # ---- constants ----
            c0 = singles.tile([P, 1], F32)
            nc.vector.memset(c0, 0.0)
            nc.const_aps.aps[(F32, 0.0)] = c0[:]
            eps_t = singles.tile([P, 1], F32)
            nc.vector.memset(eps_t, 1e-5)
            wla_sb = singles.tile([D, ROW], F32)
            nc.gpsimd.dma_start(out=wla_sb, in_=wla[:, :])
            wrb_sb = singles.tile([D, ROW + D], F32)
            nc.gpsimd.dma_start(out=wrb_sb, in_=wrb[:, :])
            attb_sb = singles.tile([P, 2 * D], F32)
            nc.gpsimd.dma_start(out=attb_sb, in_=attb[:, :])
            iotac = singles.tile([P, 1], F32)
            nc.gpsimd.dma_start(out=iotac, in_=iotac_d[:, :])
            iotar = singles.tile([P, P], F32)
            nc.gpsimd.dma_start(out=iotar, in_=iotar_d[:, :])
            ident = singles.tile([P, P], F32)
            nc.gpsimd.dma_start(out=ident, in_=ident_d[:, :])
            gb_g = gb_b = blb_sb = brb_sb = None
            if use_gb:
                gb_g = singles.tile([P, D], F32)
                nc.gpsimd.dma_start(out=gb_g, in_=gbg_d[:, :])
                gb_b = singles.tile([P, D], F32)
                nc.gpsimd.dma_start(out=gb_b, in_=gbb_d[:, :])
            if use_bl:
                blb_sb = singles.tile([P, ROW], F32)
                nc.gpsimd.dma_start(out=blb_sb, in_=blb_d[:, :])
            if use_brb:
                brb_sb = singles.tile([P, ROW + D], F32)
                nc.gpsimd.dma_start(out=brb_sb, in_=brb_d[:, :])

            res_sb = singles.tile([P, NB * D], F32)   # residual base, slot order

            ab_pools = tc.tile_pool(name="ps_t", bufs=2, space="PSUM")
            ab_pools2 = tc.tile_pool(name="ps_mm", bufs=2, space="PSUM")
            ps_t = ab_pools.__enter__()
            ps_mm = ab_pools2.__enter__()

            # ---------------- phase A: xl table for ALL nodes ----------------
            for t in range(NTILES_A):
                r0 = t * P
                r = min(P, N - r0)
                xt = io.tile([P, D], F32, tag="pa_x")
                nc.gpsimd.dma_start(out=xt[:r], in_=x_ext[r0:r0 + r, :])
                hr = _ln_relu(nc, (stat, work), xt[:r], r, use_gb, gb_g, gb_b,
                              eps=eps_t)
                hT_ps = ps_t.tile([P, P], F32, tag="pa_ht")
                nc.tensor.transpose(out=hT_ps[:, :r], in_=hr[:r],
                                    identity=ident[:r, :r])
                hTs = work.tile([P, P], F32, tag="pa_hts")
                nc.scalar.activation(out=hTs[:, :r], in_=hT_ps[:, :r],
                                     func=mybir.ActivationFunctionType.Copy)
                xl_ps = ps_mm.tile([P, ROW], F32, tag="pa_mm")
                nc.tensor.matmul(out=xl_ps[:r], lhsT=mm(hTs[:, :r]),
                                 rhs=mm(wla_sb), start=True, stop=True)
                xls = io.tile([P, ROW], F32, tag="pa_xls")
                nc.scalar.activation(out=xls[:r], in_=xl_ps[:r],
                                     func=mybir.ActivationFunctionType.Copy)
                if use_bl:
                    nc.vector.tensor_tensor(out=xls[:r], in0=xls[:r],
                                            in1=blb_sb[:r],
                                            op=mybir.AluOpType.add)
                nc.vector.memset(xls[:r, D:D + 1], 1.0)
                nc.vector.memset(xls[:r, 2 * D + 1:2 * D + 2], 1.0)
                nc.gpsimd.dma_start(out=xlt[r0:r0 + r, :], in_=xls[:r])

            # ---------------- phase B: xr + residual for local slots --------
            xp = singles.tile([P, NB * D], F32)
            pidx_sb = _load_pidx(nc, singles, pidx_d)
            for g in range(NB):
                nc.gpsimd.indirect_dma_start(
                    out=xp[:, g * D:(g + 1) * D], out_offset=None,
                    in_=x_ext[:, :],
                    in_offset=IndirectOffsetOnAxis(
                        ap=pidx_sb[:, g:g + 1], axis=0),
                )
            for g in range(NB):
                xg_ap = xp[:, g * D:(g + 1) * D]
                hr = _ln_relu(nc, (stat, work), xg_ap, P, use_gb, gb_g, gb_b,
                              eps=eps_t)
                hT_ps = ps_t.tile([P, P], F32, tag="pb_ht")
                nc.tensor.transpose(out=hT_ps, in_=hr, identity=ident)
                hTs = work.tile([P, P], F32, tag="pb_hts")
                nc.scalar.activation(out=hTs, in_=hT_ps,
                                     func=mybir.ActivationFunctionType.Copy)
                xr_ps = ps_mm.tile([P, ROW + D], F32, tag="pb_mm")
                nc.tensor.matmul(out=xr_ps, lhsT=mm(hTs), rhs=mm(wrb_sb),
                                 start=True, stop=True)
                xrt = io.tile([P, ROW], F32, tag="pb_xrt")
                nc.scalar.activation(out=xrt, in_=xr_ps[:, 0:ROW],
                                     func=mybir.ActivationFunctionType.Copy)
                if use_brb:
                    nc.vector.tensor_tensor(out=xrt, in0=xrt,
                                            in1=brb_sb[:, 0:ROW],
                                            op=mybir.AluOpType.add)
                nc.vector.memset(xrt[:, D:D + 1], 1.0)
                nc.vector.memset(xrt[:, 2 * D + 1:2 * D + 2], 1.0)
                nc.gpsimd.dma_start(out=xrl[g * P:(g + 1) * P, :], in_=xrt)
                # residual base = x_perm + h @ res_W (+ bias)
                res_ap = res_sb[:, g * D:(g + 1) * D]
                nc.vector.tensor_tensor(out=res_ap, in0=xr_ps[:, ROW:ROW + D],
                                        in1=xg_ap, op=mybir.AluOpType.add)
                if use_brb:
                    nc.vector.tensor_tensor(out=res_ap, in0=res_ap,
                                            in1=brb_sb[:, ROW:ROW + D],
                                            op=mybir.AluOpType.add)

            ab_pools2.__exit__(None, None, None)
            ab_pools.__exit__(None, None, None)

            # ---------------- phase 2: per-block edge processing -------------
            with (
                tc.tile_pool(name="gath", bufs=2) as gath,
                tc.tile_pool(name="meta", bufs=2) as meta,
                tc.tile_pool(name="blkio", bufs=2) as blkio,
                tc.tile_pool(name="chk", bufs=3) as chk,
                tc.tile_pool(name="ps_s", bufs=2, space="PSUM") as ps_s,
                tc.tile_pool(name="ps_d", bufs=2, space="PSUM") as ps_d,
                tc.tile_pool(name="ps_o", bufs=2, space="PSUM") as ps_o,
            ):
                for b in range(NB):
                    idx_sb = meta.tile([P, cpb], I32, tag="idx")
                    nc.gpsimd.dma_start(out=idx_sb, in_=gidx_d[b])
                    dc_sb = meta.tile([P, cpb], F32, tag="dc")
                    nc.gpsimd.dma_start(out=dc_sb, in_=dcol_d[b])
                    xr_sb = blkio.tile([P, ROW], F32, tag="xr")
                    nc.gpsimd.dma_start(out=xr_sb, in_=xrl[b * P:(b + 1) * P, :])
                    # HW indirect DMA consumes ONE index per out partition
                    # row, so gather chunk-by-chunk ([P,1] indices each).
                    xg = gath.tile([P, cpb * ROW], F32, tag="xg")
                    for j in range(cpb):
                        nc.gpsimd.indirect_dma_start(
                            out=xg[:, j * ROW:(j + 1) * ROW], out_offset=None,
                            in_=xlt[:, :],
                            in_offset=IndirectOffsetOnAxis(
                                ap=idx_sb[:, j:j + 1], axis=0),
                        )
                    outz = [ps_o.tile([P, D + 1], F32, tag=f"outz{h}",
                                      name=f"outz{h}")
                            for h in range(H)]
                    for j in range(cpb):
                        dstT = ps_d.tile([P, P], F32, tag="dstT")
                        nc.tensor.transpose(
                            out=dstT, in_=dc_sb[:, j:j + 1].to_broadcast([P, P]),
                            identity=ident)
                        mask = chk.tile([P, P], F32, tag="mask")
                        nc.vector.tensor_scalar(
                            out=mask, in0=dstT, scalar1=iotac, scalar2=None,
                            op0=mybir.AluOpType.is_equal)
                        s_ps = ps_s.tile([P, ROW], F32, tag="s")
                        nc.tensor.matmul(out=s_ps, lhsT=mm(mask), rhs=mm(xr_sb),
                                         start=True, stop=False)
                        nc.tensor.matmul(out=s_ps, lhsT=mm(ident),
                                         rhs=mm(xg[:, j * ROW:(j + 1) * ROW]),
                                         start=False, stop=True)
                        # lrelu(s) = 0.6*s + 0.4*|s|; att.s is linear and
                        # comes precomputed via the q columns (258, 259).
                        L = chk.tile([P, 2 * D], F32, tag="L")
                        nc.scalar.activation(
                            out=L[:, 0:D], in_=s_ps[:, 0:D],
                            func=mybir.ActivationFunctionType.Abs,
                            scale=0.4)
                        nc.scalar.activation(
                            out=L[:, D:2 * D], in_=s_ps[:, D + 1:2 * D + 1],
                            func=mybir.ActivationFunctionType.Abs,
                            scale=0.4)
                        d2 = chk.tile([P, H], F32, tag="d2")
                        wd = chk.tile([P, D], F32, tag="wd")
                        for h in range(H):
                            nc.vector.scalar_tensor_tensor(
                                out=wd, in0=L[:, h * D:(h + 1) * D],
                                scalar=1.0,
                                in1=attb_sb[:, h * D:(h + 1) * D],
                                op0=mybir.AluOpType.mult,
                                op1=mybir.AluOpType.mult,
                                accum_out=d2[:, h:h + 1])
                        alpha_t = chk.tile([P, H], F32, tag="alpha")
                        nc.vector.scalar_tensor_tensor(
                            out=alpha_t, in0=s_ps[:, 2 * D + 2:2 * D + 4],
                            scalar=0.6, in1=d2,
                            op0=mybir.AluOpType.mult, op1=mybir.AluOpType.add)
                        a_t = chk.tile([P, H], F32, tag="a")
                        nc.scalar.activation(out=a_t, in_=alpha_t,
                                             func=mybir.ActivationFunctionType.Exp)
                        for h in range(H):
                            mw = chk.tile([P, P], F32, tag=f"mw{h}")
                            nc.vector.tensor_scalar(
                                out=mw, in0=iotar, scalar1=dc_sb[:, j:j + 1],
                                scalar2=a_t[:, h:h + 1],
                                op0=mybir.AluOpType.is_equal,
                                op1=mybir.AluOpType.mult)
                            nc.tensor.matmul(
                                out=outz[h][:, :],
                                lhsT=mm(mw),
                                rhs=mm(xg[:, j * ROW + h * (D + 1):
                                          j * ROW + (h + 1) * (D + 1)]),
                                start=(j == 0), stop=(j == cpb - 1))
                    # epilogue: zr = 0.5/(z+eps); o = msg0*zr0+msg1*zr1+res
                    zs = chk.tile([P, H], F32, tag="zs")
                    for h in range(H):
                        nc.vector.tensor_scalar(
                            out=zs[:, h:h + 1],
                            in0=outz[h][:, D:D + 1],
                            scalar1=2.0, scalar2=2e-16,
                            op0=mybir.AluOpType.mult, op1=mybir.AluOpType.add)
                    zr = chk.tile([P, H], F32, tag="zr")
                    nc.vector.reciprocal(out=zr, in_=zs)
                    o_t = chk.tile([P, D], F32, tag="o1")
                    nc.vector.tensor_scalar(
                        out=o_t, in0=outz[0][:, 0:D], scalar1=zr[:, 0:1],
                        scalar2=None, op0=mybir.AluOpType.mult)
                    o2 = chk.tile([P, D], F32, tag="o2")
                    nc.vector.scalar_tensor_tensor(
                        out=o2, in0=outz[1][:, 0:D],
                        scalar=zr[:, 1:2], in1=o_t,
                        op0=mybir.AluOpType.mult, op1=mybir.AluOpType.add)
                    o3 = chk.tile([P, D], F32, tag="o3")
                    nc.vector.tensor_tensor(
                        out=o3, in0=o2, in1=res_sb[:, b * D:(b + 1) * D],
                        op=mybir.AluOpType.add)
                    nc.gpsimd.dma_start(out=out_ext[b * P:(b + 1) * P, :], in_=o3)

    return nc


def _load_pidx(nc, singles, pidx_d):
    pidx_sb = singles.tile([P, NB], I32, name="pidx_sb")
    nc.gpsimd.dma_start(out=pidx_sb, in_=pidx_d[:, :])
    return pidx_sb[:, :]


# ----------------------------------------------------------------------------
# BIR legalization: this container's walrus only accepts sem waits on
# standalone EventSemaphore instructions (raw-bass style); inline on_wait on
# compute/DMA instructions dies with "Too many sync wait commands". Hoist
# every inline wait into its own EventSemaphore right before the instruction.
# ----------------------------------------------------------------------------

def _legalize_waits(bir_bytes: bytes) -> bytes:
    import json

    d = json.loads(bir_bytes)

    # Semaphores that are ever decremented can't use the monotonic-dedupe.
    dec_sems = set()
    for f in d.get("functions", []):
        for b in f.get("blocks", []):
            for i in b.get("instructions", []):
                si = i.get("sync_info")
                if not si:
                    continue
                for u in si.get("on_update") or []:
                    if u.get("update_mode") in ("sem-dec", "sem-sub-imm"):
                        dec_sems.add(u.get("id"))

    uid = [0]
    for f in d.get("functions", []):
        for b in f.get("blocks", []):
            new = []
            # (engine, sem id) -> max wait value already enforced
            seen = {}
            for i in b.get("instructions", []):
                si = i.get("sync_info")
                waits = (si.get("on_wait") or []) if si else []
                if si and waits and i.get("opcode") != "EventSemaphore":
                    eng = i.get("engine")
                    for w in waits:
                        sem = w.get("id")
                        mode = w.get("wait_mode")
                        val = w.get("wait_value", 0)
                        key = (eng, sem)
                        if (
                            mode == "sem-ge-imm"
                            and sem not in dec_sems
                            and seen.get(key, -1) >= val
                        ):
                            continue
                        uid[0] += 1
                        new.append({
                            "debug": i.get("debug", 0),
                            "engine": eng,
                            "ins": [],
                            "name": f"lw{uid[0]}-{i.get('name', 'i')}",
                            "opcode": "EventSemaphore",
                            "outs": [],
                            "sync_info": {"on_update": [], "on_wait": [w]},
                        })
                        if mode == "sem-ge-imm" and sem not in dec_sems:
                            seen[key] = max(seen.get(key, -1), val)
                    si["on_wait"] = []
                # updates can move sems; conservatively only invalidate when
                # this instruction decrements something (loops aside, sems
                # here are monotonic). Nothing to do for increments.
                new.append(i)
            b["instructions"] = new
    return json.dumps(d).encode()


def _install_legalizer(nc):
    orig = nc.to_json_bytes

    def patched():
        return _legalize_waits(orig())

    nc.to_json_bytes = patched
    return nc


# ----------------------------------------------------------------------------
# Host entry point
# ----------------------------------------------------------------------------

_CACHE = {}
LAST_RESULTS = None


def kernel(x, edge_index, gamma, beta, Wl, bl, Wr, br, att, res_W, bias):
    x = np.asarray(x, dtype=np.float32)
    edge_index = np.asarray(edge_index)
    gamma = np.asarray(gamma, dtype=np.float32)
    beta = np.asarray(beta, dtype=np.float32)
    Wl = np.asarray(Wl, dtype=np.float32)
    bl = np.asarray(bl, dtype=np.float32)
    Wr = np.asarray(Wr, dtype=np.float32)
    br = np.asarray(br, dtype=np.float32)
    att = np.asarray(att, dtype=np.float32)
    res_W = np.asarray(res_W, dtype=np.float32)
    bias = np.asarray(bias, dtype=np.float32)

    cpb, cores = _pack_graph(edge_index)

    use_gb = not (np.all(gamma == 1.0) and np.all(beta == 0.0))
    use_bl = bool(np.any(bl != 0.0))
    use_brb = bool(np.any(br != 0.0) or np.any(bias != 0.0))

    key = (cpb, use_gb, use_bl, use_brb)
    if key not in _CACHE:
        _CACHE[key] = _install_legalizer(
            build_program(cpb, use_gb, use_bl, use_brb))
    nc = _CACHE[key]

    # Augmented weights: cols [h0(128) | 0 | h1(128) | 0 | att0.Wh0 | att1.Wh1]
    def augment(W):  # [D, 2D] -> [D, ROW]
        Wa = np.zeros((D, ROW), dtype=np.float32)
        Wa[:, 0:D] = W[:, 0:D]
        Wa[:, D + 1:2 * D + 1] = W[:, D:2 * D]
        Wa[:, 2 * D + 2] = W[:, 0:D] @ att[0]
        Wa[:, 2 * D + 3] = W[:, D:2 * D] @ att[1]
        return Wa

    wla = augment(Wl)
    wrb = np.zeros((D, ROW + D), dtype=np.float32)
    wrb[:, 0:ROW] = augment(Wr)
    wrb[:, ROW:ROW + D] = res_W
    attb = np.tile(att.reshape(1, 2 * D), (P, 1)).astype(np.float32)
    iotac = np.arange(P, dtype=np.float32).reshape(P, 1)
    iotar = np.tile(np.arange(P, dtype=np.float32).reshape(1, P), (P, 1))
    ident = np.eye(P, dtype=np.float32)

    base = dict(x=x, wla=wla, wrb=wrb, attb=attb, iotac=iotac,
                iotar=iotar, ident=np.ascontiguousarray(ident))
    if use_gb:
        base["gbg"] = np.tile(gamma.reshape(1, D), (P, 1)).astype(np.float32)
        base["gbb"] = np.tile(beta.reshape(1, D), (P, 1)).astype(np.float32)
    if use_bl:
        blb = np.zeros((P, ROW), dtype=np.float32)
        blb[:, 0:D] = bl[0:D]
        blb[:, D + 1:2 * D + 1] = bl[D:2 * D]
        blb[:, 2 * D + 2] = float(att[0] @ bl[0:D])
        blb[:, 2 * D + 3] = float(att[1] @ bl[D:2 * D])
        base["blb"] = blb
    if use_brb:
        brb = np.zeros((P, ROW + D), dtype=np.float32)
        brb[:, 0:D] = br[0:D]
        brb[:, D + 1:2 * D + 1] = br[D:2 * D]
        brb[:, 2 * D + 2] = float(att[0] @ br[0:D])
        brb[:, 2 * D + 3] = float(att[1] @ br[D:2 * D])
        brb[:, ROW:ROW + D] = bias
        base["brb"] = brb

    in_maps = []
    for c in range(N_CORES):
        m = dict(base)
        m["gidx"] = cores[c]["gidx"]
        m["dcol"] = cores[c]["dcol"]
        m["pidx"] = cores[c]["pidx"]
        in_maps.append(m)

    trace = bool(int(os.environ.get("GAT_TRACE", "0")))
    res = run_bass_kernel_spmd(nc, in_maps, list(range(N_CORES)), trace=trace)
    global LAST_RESULTS
    LAST_RESULTS = res

    out = np.zeros((N, D), dtype=np.float32)
    for c in range(N_CORES):
        oc = res.results[c]["out"]          # [SLOTS, D]
        ph = cores[c]["pidx_host"]          # [NB, P]
        vd = cores[c]["valid"]              # [NB, P]
        oc = oc.reshape(NB, P, D)
        out[ph[vd]] = oc[vd]
    return out
